# revision 7
# baseline (speedup 1.0000x reference)
"""Batched Householder reflection: s_new[b] = s[b] - 2*(v[b]@s[b])/(v[b]@v[b]) * v[b].

Full inputs v, s: [512, 512] f32. Sharded batch-parallel across 8 NeuronCores
(64 rows per core). Per core: rows on SBUF partitions, K=512 on the free axis.
v and s shards are stacked host-side into one [2, 64, 512] DRAM tensor.

v3 schedule (perfetto-driven):
- Load phase is desc-gen + doorbell bound, not bandwidth bound (a queue's
  descriptors fan out over ~16 DMA engines at ~160-320 GB/s). SP has the
  fastest/most predictable desc-gen (~29ns/line), ACT's is erratic (up to
  ~42ns/line), GpSimd's Q7 starts ~700ns late and serializes its queues.
  So: SP gens v[0:44] then s[0:36]; pool s (one SWDGE ring: gpsimd.dma_start is pinned to queue 0); ACT gens
  NOTHING on the load side so its Square table load (1283ns, triggered by
  the warm op) runs at the main-BB barrier and finishes by ~8.2us, well
  before v lands.
- ACT square waits only v (sv>=32); DVE dot waits v+s; coef/final chain on
  DVE. Two semaphores: sv = v-loads + DVE chain + stores, ss = s-loads +
  nsq visibility.
  dotm2 = rowsum(-2*v*s)   (DVE stt accum_out)
  nsq   = rowsum(v*v)      (ACT Square accum_out, parallel with dot)
  coef  = dotm2 * (1/nsq)  (DVE reciprocal + stt)
  out   = coef*v + s       (DVE stt, per-partition scalar broadcast)
- Stores: SP 26 / ACT 14 / pool 24 rows (ACT smallest: slow desc-gen).
- VE clears ss right after final (its waiters provably passed); SP waits
  sv>=84 (3 stores done) and clears sv.
"""

import numpy as np

B, K = 512, 512
N_CORES = 8
B_LOC = B // N_CORES  # 64 rows per core

_nc = None


def _build():
    import concourse.bass as bass
    from concourse import mybir

    nc = bass.Bass("TRN2", debug=False, num_devices=N_CORES, num_swdge_queues=1)
    f32 = mybir.dt.float32

    vs = nc.dram_tensor("vs", [2, B_LOC, K], f32, kind="ExternalInput").ap()
    out = nc.dram_tensor("out", [B_LOC, K], f32, kind="ExternalOutput").ap()

    vs_t = nc.alloc_sbuf_tensor("vs_t", [B_LOC, 2, K], f32).ap()
    o_t = nc.alloc_sbuf_tensor("o_t", [B_LOC, K], f32).ap()
    junk_vs = nc.alloc_sbuf_tensor("junk_vs", [B_LOC, K], f32).ap()
    junk_vv = nc.alloc_sbuf_tensor("junk_vv", [B_LOC, K], f32).ap()
    warm = nc.alloc_sbuf_tensor("warm", [B_LOC, 1], f32).ap()
    dotm2 = nc.alloc_sbuf_tensor("dotm2", [B_LOC, 1], f32).ap()
    nsq = nc.alloc_sbuf_tensor("nsq", [B_LOC, 1], f32).ap()
    coef = nc.alloc_sbuf_tensor("coef", [B_LOC, 1], f32).ap()
    rcp = nc.alloc_sbuf_tensor("rcp", [B_LOC, 1], f32).ap()

    sv = nc.alloc_semaphore("sv")
    ss = nc.alloc_semaphore("ss")

    mult = mybir.AluOpType.mult
    add = mybir.AluOpType.add
    Square = mybir.ActivationFunctionType.Square

    sp, act, ve, pl = nc.sync, nc.scalar, nc.vector, nc.gpsimd
    v_t = vs_t[:, 0, :]
    s_t = vs_t[:, 1, :]
    zero64 = nc.const_aps.scalar_like(0.0, dotm2[:])

    # ---- ACT: warm fires the Square table load immediately at the barrier
    act.activation(out=warm[:], in_=zero64, func=Square)

    # ---- loads ----
    VTOP, STOP = 44, 36  # SP's shares; pool q0/q1 take the bottoms
    sp.dma_start(out=vs_t[:VTOP, 0, :], in_=vs[0, :VTOP, :]).then_inc(sv, 16)
    sp.dma_start(out=vs_t[VTOP:, 0, :], in_=vs[0, VTOP:, :]).then_inc(sv, 16)
    pl.dma_start(out=vs_t[:STOP, 1, :], in_=vs[1, :STOP, :]).then_inc(ss, 16)
    pl.dma_start(out=vs_t[STOP:, 1, :], in_=vs[1, STOP:, :]).then_inc(ss, 16)

    # nsq = rowsum(v*v) on ACT as soon as v lands
    act.wait_ge(sv, 32)
    act.activation(out=junk_vv[:], in_=v_t, func=Square, accum_out=nsq[:]).then_inc(
        ss, 1
    )

    # DVE chain; sv also tracks DVE write visibility
    ve.wait_ge(sv, 32)
    ve.wait_ge(ss, 32)
    ve.scalar_tensor_tensor(
        out=junk_vs[:],
        in0=v_t,
        scalar=-2.0,
        in1=s_t,
        op0=mult,
        op1=mult,
        accum_out=dotm2[:],
    ).then_inc(sv, 1)
    ve.wait_ge(ss, 33)
    ve.reciprocal(out=rcp[:], in_=nsq[:]).then_inc(sv, 1)
    ve.wait_ge(sv, 34)
    ve.scalar_tensor_tensor(
        out=coef[:], in0=dotm2[:], scalar=1.0, in1=rcp[:], op0=mult, op1=mult
    ).then_inc(sv, 1)
    ve.wait_ge(sv, 35)
    ve.scalar_tensor_tensor(
        out=o_t[:],
        in0=v_t,
        scalar=coef[:],
        in1=s_t,
        op0=mult,
        op1=add,
    ).then_inc(sv, 1)
    # ss's waiters (ACT square, VE reciprocal) have provably passed
    ve.sem_clear(ss)

    # ---- stores: three streams (SP / ACT / Pool q0) ----
    sp.wait_ge(sv, 36)
    sp.dma_start(out=out[0:26, :], in_=o_t[0:26, :]).then_inc(sv, 16)
    act.wait_ge(sv, 36)
    act.dma_start(out=out[26:40, :], in_=o_t[26:40, :]).then_inc(sv, 16)
    pl.wait_ge(sv, 36)
    pl.dma_start(out=out[40:64, :], in_=o_t[40:64, :]).then_inc(sv, 16)

    # SP resets sv for re-execution (PJRT reuses the loaded NEFF).
    # sv=84 proves every waiter (ACT/Pool store waits included) has passed.
    sp.wait_ge(sv, 84)
    sp.sem_clear(sv)

    return nc


def kernel(i=None, v=None, s=None, **_):
    global _nc
    from concourse.bass_utils import run_bass_kernel_spmd

    if _nc is None:
        _nc = _build()

    v = np.asarray(v, dtype=np.float32)
    s = np.asarray(s, dtype=np.float32)
    in_maps = [
        {
            "vs": np.ascontiguousarray(
                np.stack(
                    [v[c * B_LOC : (c + 1) * B_LOC], s[c * B_LOC : (c + 1) * B_LOC]]
                )
            )
        }
        for c in range(N_CORES)
    ]
    res = run_bass_kernel_spmd(_nc, in_maps, core_ids=list(range(N_CORES)))
    return np.concatenate([r["out"] for r in res.results], axis=0)


# revision 10
# speedup vs baseline: 1.0930x; 1.0930x over previous
"""Batched Householder reflection: s_new[b] = s[b] - 2*(v[b]@s[b])/(v[b]@v[b]) * v[b].

Full inputs v, s: [512, 512] f32. Sharded batch-parallel across 8 NeuronCores
(64 rows per core). Per core: rows on SBUF partitions, K=512 on the free axis.
v and s shards are stacked host-side into one [2, 64, 512] DRAM tensor.

v5 schedule (perfetto-driven, refines the v1 structure):
- 4 load streams: SP v[0:40], ACT s[0:40], pool v[40:64] THEN s[40:64]
  (pool's two dma_starts share one SWDGE ring and serialize; v goes first
  so ACT's Square starts earlier; SP/ACT carry more lines since their
  HWDGE desc-gen is parallel).
- ACT order: load desc-gen, then a warm Square on const zeros. The warm
  pulls the 1283ns ACT table load into the load window; the real square
  then starts as soon as v lands (sv>=32), accum_out -> nsq.
- DVE: dot (with -2 folded via op0 scalar), reciprocal, coef, final.
  3 semaphores: sv (v-loads + DVE chain), ss (s-loads + nsq visibility),
  so (store completions). Stores: SP 28 / ACT 12 / pool 24 (ACT's store
  desc-gen is the slowest, pool's starts latest).
- SP waits so>=48 (all stores landed, which also proves ACT/pool passed
  their sv-waits) then clears all three sems for NEFF re-execution.
"""

import numpy as np

B, K = 512, 512
N_CORES = 8
B_LOC = B // N_CORES  # 64 rows per core

_nc = None


def _build():
    import concourse.bass as bass
    from concourse import mybir

    nc = bass.Bass("TRN2", debug=False, num_devices=N_CORES, num_swdge_queues=1)
    f32 = mybir.dt.float32

    vs = nc.dram_tensor("vs", [2, B_LOC, K], f32, kind="ExternalInput").ap()
    out = nc.dram_tensor("out", [B_LOC, K], f32, kind="ExternalOutput").ap()

    vs_t = nc.alloc_sbuf_tensor("vs_t", [B_LOC, 2, K], f32).ap()
    o_t = nc.alloc_sbuf_tensor("o_t", [B_LOC, K], f32).ap()
    junk_vs = nc.alloc_sbuf_tensor("junk_vs", [B_LOC, K], f32).ap()
    junk_vv = nc.alloc_sbuf_tensor("junk_vv", [B_LOC, K], f32).ap()
    warm = nc.alloc_sbuf_tensor("warm", [B_LOC, 1], f32).ap()
    dotm2 = nc.alloc_sbuf_tensor("dotm2", [B_LOC, 1], f32).ap()
    nsq = nc.alloc_sbuf_tensor("nsq", [B_LOC, 1], f32).ap()
    coef = nc.alloc_sbuf_tensor("coef", [B_LOC, 1], f32).ap()
    rcp = nc.alloc_sbuf_tensor("rcp", [B_LOC, 1], f32).ap()

    sv = nc.alloc_semaphore("sv")
    ss = nc.alloc_semaphore("ss")
    so = nc.alloc_semaphore("so")

    mult = mybir.AluOpType.mult
    add = mybir.AluOpType.add
    Square = mybir.ActivationFunctionType.Square

    sp, act, ve, pl = nc.sync, nc.scalar, nc.vector, nc.gpsimd
    v_t = vs_t[:, 0, :]
    s_t = vs_t[:, 1, :]
    zero64 = nc.const_aps.scalar_like(0.0, dotm2[:])

    # ---- loads: v first everywhere ----
    SPL = 40
    sp.dma_start(out=vs_t[:SPL, 0, :], in_=vs[0, :SPL, :]).then_inc(sv, 16)
    act.dma_start(out=vs_t[:SPL, 1, :], in_=vs[1, :SPL, :]).then_inc(ss, 16)
    act.activation(out=warm[:], in_=zero64, func=Square)  # pulls table load early
    pl.dma_start(out=vs_t[SPL:, 0, :], in_=vs[0, SPL:, :]).then_inc(sv, 16)
    pl.dma_start(out=vs_t[SPL:, 1, :], in_=vs[1, SPL:, :]).then_inc(ss, 16)

    # nsq = rowsum(v*v) on ACT as soon as v lands
    act.wait_ge(sv, 32)
    act.activation(out=junk_vv[:], in_=v_t, func=Square, accum_out=nsq[:]).then_inc(
        ss, 1
    )

    # DVE chain; sv also tracks DVE write visibility
    ve.wait_ge(sv, 32)
    ve.wait_ge(ss, 32)
    ve.scalar_tensor_tensor(
        out=junk_vs[:],
        in0=v_t,
        scalar=-2.0,
        in1=s_t,
        op0=mult,
        op1=mult,
        accum_out=dotm2[:],
    ).then_inc(sv, 1)
    ve.wait_ge(ss, 33)
    ve.reciprocal(out=rcp[:], in_=nsq[:]).then_inc(sv, 1)
    ve.wait_ge(sv, 34)
    ve.scalar_tensor_tensor(
        out=coef[:], in0=dotm2[:], scalar=1.0, in1=rcp[:], op0=mult, op1=mult
    ).then_inc(sv, 1)
    ve.wait_ge(sv, 35)
    ve.scalar_tensor_tensor(
        out=o_t[:],
        in0=v_t,
        scalar=coef[:],
        in1=s_t,
        op0=mult,
        op1=add,
    ).then_inc(sv, 1)

    # ---- stores ----
    sp.wait_ge(sv, 36)
    sp.dma_start(out=out[0:28, :], in_=o_t[0:28, :]).then_inc(so, 16)
    act.wait_ge(sv, 36)
    act.dma_start(out=out[28:40, :], in_=o_t[28:40, :]).then_inc(so, 16)
    pl.wait_ge(sv, 36)
    pl.dma_start(out=out[40:64, :], in_=o_t[40:64, :]).then_inc(so, 16)

    # so=48: all stores landed => ACT/pool passed their sv-waits too.
    sp.wait_ge(so, 48)
    sp.sem_clear(sv)
    sp.sem_clear(ss)
    sp.sem_clear(so)

    return nc


def kernel(i=None, v=None, s=None, **_):
    global _nc
    from concourse.bass_utils import run_bass_kernel_spmd

    if _nc is None:
        _nc = _build()

    v = np.asarray(v, dtype=np.float32)
    s = np.asarray(s, dtype=np.float32)
    in_maps = [
        {
            "vs": np.ascontiguousarray(
                np.stack(
                    [v[c * B_LOC : (c + 1) * B_LOC], s[c * B_LOC : (c + 1) * B_LOC]]
                )
            )
        }
        for c in range(N_CORES)
    ]
    res = run_bass_kernel_spmd(_nc, in_maps, core_ids=list(range(N_CORES)))
    return np.concatenate([r["out"] for r in res.results], axis=0)


# revision 11
# speedup vs baseline: 1.1768x; 1.0767x over previous
"""Batched Householder reflection: s_new[b] = s[b] - 2*(v[b]@s[b])/(v[b]@v[b]) * v[b].

Full inputs v, s: [512, 512] f32. Sharded batch-parallel across 8 NeuronCores
(64 rows per core). Per core: rows on SBUF partitions, K=512 on the free axis.
v and s shards are stacked host-side into one [2, 64, 512] DRAM tensor.

v5 schedule (perfetto-driven, refines the v1 structure):
- 4 load streams: SP v[0:40], ACT s[0:40], pool v[40:64] THEN s[40:64]
  (pool's two dma_starts share one SWDGE ring and serialize; v goes first
  so ACT's Square starts earlier; SP/ACT carry more lines since their
  HWDGE desc-gen is parallel).
- ACT order: load desc-gen, then a warm Square on const zeros. The warm
  pulls the 1283ns ACT table load into the load window; the real square
  then starts as soon as v lands (sv>=32), accum_out -> nsq.
- DVE: dot (with -2 folded via op0 scalar), reciprocal, coef, final.
  3 semaphores: sv (v-loads + DVE chain), ss (s-loads + nsq visibility),
  so (store completions). Stores: SP 28 / ACT 12 / pool 24 (ACT's store
  desc-gen is the slowest, pool's starts latest).
- SP waits so>=48 (all stores landed, which also proves ACT/pool passed
  their sv-waits) then clears all three sems for NEFF re-execution.
"""

import numpy as np

B, K = 512, 512
N_CORES = 8
B_LOC = B // N_CORES  # 64 rows per core

_nc = None


def _build():
    import concourse.bass as bass
    from concourse import mybir

    nc = bass.Bass("TRN2", debug=False, num_devices=N_CORES, num_swdge_queues=1)
    f32 = mybir.dt.float32

    vs = nc.dram_tensor("vs", [2, B_LOC, K], f32, kind="ExternalInput").ap()
    out = nc.dram_tensor("out", [B_LOC, K], f32, kind="ExternalOutput").ap()

    vs_t = nc.alloc_sbuf_tensor("vs_t", [B_LOC, 2, K], f32).ap()
    o_t = nc.alloc_sbuf_tensor("o_t", [B_LOC, K], f32).ap()
    junk_vs = nc.alloc_sbuf_tensor("junk_vs", [B_LOC, K], f32).ap()
    junk_vv = nc.alloc_sbuf_tensor("junk_vv", [B_LOC, K], f32).ap()
    warm = nc.alloc_sbuf_tensor("warm", [B_LOC, 1], f32).ap()
    dotm2 = nc.alloc_sbuf_tensor("dotm2", [B_LOC, 1], f32).ap()
    nsq = nc.alloc_sbuf_tensor("nsq", [B_LOC, 1], f32).ap()
    coef = nc.alloc_sbuf_tensor("coef", [B_LOC, 1], f32).ap()
    rcp = nc.alloc_sbuf_tensor("rcp", [B_LOC, 1], f32).ap()

    sv = nc.alloc_semaphore("sv")
    ss = nc.alloc_semaphore("ss")
    so = nc.alloc_semaphore("so")

    mult = mybir.AluOpType.mult
    add = mybir.AluOpType.add
    Square = mybir.ActivationFunctionType.Square

    sp, act, ve, pl = nc.sync, nc.scalar, nc.vector, nc.gpsimd
    v_t = vs_t[:, 0, :]
    s_t = vs_t[:, 1, :]
    zero64 = nc.const_aps.scalar_like(0.0, dotm2[:])

    # ---- loads: v first everywhere. ACT capped at 32 lines (its HWDGE
    # desc-gen falls off a cliff above ~32: 683ns@32 vs 1682ns@40, which
    # also pushes the warm-triggered table load past v-arrival). The other
    # 32 s-rows ride as small second dma_starts on SP and pool. ----
    sp.dma_start(out=vs_t[:40, 0, :], in_=vs[0, :40, :]).then_inc(sv, 16)
    sp.dma_start(out=vs_t[32:48, 1, :], in_=vs[1, 32:48, :]).then_inc(ss, 16)
    act.dma_start(out=vs_t[:32, 1, :], in_=vs[1, :32, :]).then_inc(ss, 16)
    act.activation(out=warm[:], in_=zero64, func=Square)  # pulls table load early
    pl.dma_start(out=vs_t[40:, 0, :], in_=vs[0, 40:, :]).then_inc(sv, 16)
    pl.dma_start(out=vs_t[48:, 1, :], in_=vs[1, 48:, :]).then_inc(ss, 16)

    # nsq = rowsum(v*v) on ACT as soon as v lands
    act.wait_ge(sv, 32)
    act.activation(out=junk_vv[:], in_=v_t, func=Square, accum_out=nsq[:]).then_inc(
        ss, 1
    )

    # DVE chain; sv also tracks DVE write visibility
    ve.wait_ge(sv, 32)
    ve.wait_ge(ss, 48)
    ve.scalar_tensor_tensor(
        out=junk_vs[:],
        in0=v_t,
        scalar=-2.0,
        in1=s_t,
        op0=mult,
        op1=mult,
        accum_out=dotm2[:],
    ).then_inc(sv, 1)
    ve.wait_ge(ss, 49)
    ve.reciprocal(out=rcp[:], in_=nsq[:]).then_inc(sv, 1)
    ve.wait_ge(sv, 34)
    ve.scalar_tensor_tensor(
        out=coef[:], in0=dotm2[:], scalar=1.0, in1=rcp[:], op0=mult, op1=mult
    ).then_inc(sv, 1)
    ve.wait_ge(sv, 35)
    ve.scalar_tensor_tensor(
        out=o_t[:],
        in0=v_t,
        scalar=coef[:],
        in1=s_t,
        op0=mult,
        op1=add,
    ).then_inc(sv, 1)

    # ---- stores ----
    sp.wait_ge(sv, 36)
    sp.dma_start(out=out[0:28, :], in_=o_t[0:28, :]).then_inc(so, 16)
    act.wait_ge(sv, 36)
    act.dma_start(out=out[28:40, :], in_=o_t[28:40, :]).then_inc(so, 16)
    pl.wait_ge(sv, 36)
    pl.dma_start(out=out[40:64, :], in_=o_t[40:64, :]).then_inc(so, 16)

    # so=48: all stores landed => ACT/pool passed their sv-waits too.
    sp.wait_ge(so, 48)
    sp.sem_clear(sv)
    sp.sem_clear(ss)
    sp.sem_clear(so)

    return nc


def kernel(i=None, v=None, s=None, **_):
    global _nc
    from concourse.bass_utils import run_bass_kernel_spmd

    if _nc is None:
        _nc = _build()

    v = np.asarray(v, dtype=np.float32)
    s = np.asarray(s, dtype=np.float32)
    in_maps = [
        {
            "vs": np.ascontiguousarray(
                np.stack(
                    [v[c * B_LOC : (c + 1) * B_LOC], s[c * B_LOC : (c + 1) * B_LOC]]
                )
            )
        }
        for c in range(N_CORES)
    ]
    res = run_bass_kernel_spmd(_nc, in_maps, core_ids=list(range(N_CORES)))
    return np.concatenate([r["out"] for r in res.results], axis=0)
